# revision 5
# baseline (speedup 1.0000x reference)
"""Trainium2 Bass kernel for CurvSelfAttention (v2).

Reference computation (per batch b):
    Q = hs @ Wq + bq ; K = hs @ Wk + bk ; V = hs @ Wv + bv      # [S, H]
    s = sigmoid(hs @ Ws + bs) * 0.2 + 0.9                        # [S, NH*G]
    Q[:, h*64+g*8+r] *= s[:, h*8+g]
    per head h: ctx_h = softmax(Q_h K_h^T / 8) V_h               # [S, 64]
    out = concat_h(ctx_h)                                        # [S, NH*64]

Sharding over 8 cores: core c = (b = c // 2, hh = c % 2); each core owns
batch b and heads hh*8 .. hh*8+8 (512 output columns). No collectives.

v2 schedule (vs v1): the kernel is ScalarE(exp)-bound, so everything is
arranged around one big 2048-wide ACT per (head-pair, t-chunk):
  - scores for BOTH heads of a pair land in one 4-bank PSUM tile
    [128, 2048] (quarters written by 4 matmuls, two concurrent per 64-row
    half of the PE array), drained by a single Exp ACTIVATE -> bf16 ring.
  - ctx accumulates [65, 1024] per head in the other 4 PSUM banks, lagging
    the probs ring by LAG chunks; V carries a ones column so row 64 is the
    softmax denominator.
  - ctx output is copied to bf16 and transposed by the DMA xbar engine
    (not TensorE), then normalized on VectorE (reciprocal of row 64).
  - Q/K projections for pairs 1-3 drain as filler units inside the
    attention loop (PSUM tags time-share the ctx banks); V + pair-0 Q/K +
    the dynamic scale run in a prefix whose PSUM pool is released before
    the attention pools allocate.
"""

import os
import sys

sys.path.insert(0, "/opt/trn_rl_repo")

import numpy as np
import ml_dtypes
from collections import deque
from contextlib import ExitStack

import concourse.bass as bass
import concourse.bacc as bacc
import concourse.tile as tile
from concourse import mybir
from concourse import bass_utils

F32 = mybir.dt.float32
BF16 = mybir.dt.bfloat16
AF = mybir.ActivationFunctionType
ALU = mybir.AluOpType

P = 128          # SBUF partitions
NB = 512         # matmul moving free-dim block
W2 = 1024        # attention s-window per head (one ctx psum row)
HD = 64          # head dim
G = 8            # groups per head
RING = 6         # probs ring chunks
LAG = 2          # ctx trails exp by this many chunks
SC_MIN, SC_MAX = 0.9, 1.1


def build_bass(S=2048, H=1024, NHL=8):
    """Build the per-core Bass module. NHL = local heads; JL = NHL*64."""
    JL = NHL * HD
    GL = NHL * G           # compact scale channels
    KT = H // P            # contraction k-tiles
    JB = JL // P           # j row-blocks for Q/K (2 heads each)
    NTB = S // NB          # 512-wide t blocks
    NTC = S // P           # 128-wide t chunks
    NSSB = S // W2         # s-windows per head
    HP = NHL // 2          # head pairs

    nc = bacc.Bacc(trn_type="TRN2", target_bir_lowering=False, debug=False,
                   num_devices=8)

    hs = nc.dram_tensor("hs", [S, H], BF16, kind="ExternalInput").ap()
    wq = nc.dram_tensor("wq", [H, JL], BF16, kind="ExternalInput").ap()
    wk = nc.dram_tensor("wk", [H, JL], BF16, kind="ExternalInput").ap()
    wv = nc.dram_tensor("wv", [H, JL], BF16, kind="ExternalInput").ap()
    ws = nc.dram_tensor("ws", [H, GL], BF16, kind="ExternalInput").ap()
    bq = nc.dram_tensor("bq", [JL], F32, kind="ExternalInput").ap()
    bk = nc.dram_tensor("bk", [JL], F32, kind="ExternalInput").ap()
    bv = nc.dram_tensor("bv", [JL], F32, kind="ExternalInput").ap()
    bs = nc.dram_tensor("bs", [GL], F32, kind="ExternalInput").ap()
    out = nc.dram_tensor("out", [S, JL], F32, kind="ExternalOutput").ap()
    sxd = nc.dram_tensor("sxd", [GL, S], BF16, kind="Internal").ap()

    with tile.TileContext(nc) as tc, ExitStack() as ctx:
        cpool = ctx.enter_context(tc.tile_pool(name="consts", bufs=1))
        qkpool = ctx.enter_context(tc.tile_pool(name="qk", bufs=1))
        vpool = ctx.enter_context(tc.tile_pool(name="v", bufs=1))
        sxpool = ctx.enter_context(tc.tile_pool(name="sexp", bufs=1))
        hpool = ctx.enter_context(tc.tile_pool(name="hsT", bufs=1))
        wpool = ctx.enter_context(tc.tile_pool(name="wts", bufs=1))
        ptmp = ctx.enter_context(tc.tile_pool(name="ptmp", bufs=2))

        # ---- constants (small DMAs on the SW DGE queue) ----
        bq_sb = []
        bk_sb = []
        for jb in range(JB):
            t = cpool.tile([P, 1], F32, tag=f"bq{jb}")
            nc.gpsimd.dma_start(
                t[:], bq[jb * P:(jb + 1) * P].rearrange("(a b) -> a b", b=1))
            bq_sb.append(t)
            t = cpool.tile([P, 1], F32, tag=f"bk{jb}")
            nc.gpsimd.dma_start(
                t[:], bk[jb * P:(jb + 1) * P].rearrange("(a b) -> a b", b=1))
            bk_sb.append(t)
        bs_sb = cpool.tile([GL, 1], F32, tag="bs")
        nc.gpsimd.dma_start(bs_sb[:], bs.rearrange("(a b) -> a b", b=1))
        bvb = cpool.tile([P, JL], F32, tag="bvb")
        nc.gpsimd.dma_start(
            bvb[:], bv.rearrange("(a b) -> a b", a=1).broadcast_to([P, JL]))

        # persistent activation tensors
        q_sb = [qkpool.tile([P, S], BF16, tag=f"q{jb}", name=f"q{jb}")
                for jb in range(JB)]
        k_sb = [qkpool.tile([P, S], BF16, tag=f"k{jb}", name=f"k{jb}")
                for jb in range(JB)]
        # V as [t-chunk][128, NHL, 65]; col 64 of each head = ones (denom)
        v_sb = [vpool.tile([P, NHL, HD + 1], BF16, tag=f"v{tc_}", name=f"v{tc_}")
                for tc_ in range(NTC)]
        sexp = [sxpool.tile([P, S], BF16, tag=f"sx{jb}", name=f"sx{jb}")
                for jb in range(JB)]

        # ---- input loads: ws first, hsT transposes, then weights ----
        ws_sb = []
        for k in range(KT):
            t = wpool.tile([P, GL], BF16, tag=f"ws{k}", name=f"ws{k}")
            nc.sync.dma_start(t[:], ws[k * P:(k + 1) * P, :])
            ws_sb.append(t)
        hsT = []
        for k in range(KT):
            t = hpool.tile([P, S], BF16, tag=f"hsT{k}", name=f"hsT{k}")
            nc.sync.dma_start_transpose(t[:], hs[:, k * P:(k + 1) * P])
            hsT.append(t)
        wq_sb, wk_sb, wv_sb = [], [], []
        for k in range(KT):
            for name, dram, lst in (("wv", wv, wv_sb), ("wq", wq, wq_sb),
                                    ("wk", wk, wk_sb)):
                t = wpool.tile([P, JL], BF16, tag=f"{name}{k}",
                               name=f"{name}{k}")
                nc.sync.dma_start(t[:], dram[k * P:(k + 1) * P, :])
                lst.append(t)

        # ---- projection work units ----
        # deferred Q/K half-units (4 matmuls each) keyed for the attention
        # loop's filler slots; V and pair-0 Q/K run in the prefix.
        open_ps = {}

        def emit_qk_unit(jb, kind, tb, half, pool, tag):
            wlist = wq_sb if kind == "q" else wk_sb
            key = (jb, kind, tb)
            if half == 0:
                open_ps[key] = pool.tile([P, NB], F32, tag=tag, name="pp")
            ps = open_ps[key]
            kh = KT // 2
            for k in range(kh * half, kh * half + kh):
                nc.tensor.matmul(
                    ps[:], wlist[k][:, jb * P:(jb + 1) * P],
                    hsT[k][:, tb * NB:(tb + 1) * NB],
                    start=(k == 0), stop=(k == KT - 1))
            if half == 1:
                del open_ps[key]
                if kind == "q":
                    nc.vector.scalar_tensor_tensor(
                        q_sb[jb][:, tb * NB:(tb + 1) * NB], ps[:],
                        bq_sb[jb][:], sexp[jb][:, tb * NB:(tb + 1) * NB],
                        ALU.add, ALU.mult)
                else:
                    nc.vector.tensor_scalar_add(
                        k_sb[jb][:, tb * NB:(tb + 1) * NB], ps[:], bk_sb[jb][:])

        def emit_v_unit(tc_, half, pool, tag):
            key = ("v", tc_)
            if half == 0:
                open_ps[key] = pool.tile([P, JL], F32, tag=tag, name="pp")
            ps = open_ps[key]
            kh = KT // 2
            for k in range(kh * half, kh * half + kh):
                nc.tensor.matmul(
                    ps[:], hsT[k][:, tc_ * P:(tc_ + 1) * P], wv_sb[k][:],
                    start=(k == 0), stop=(k == KT - 1))
            if half == 1:
                del open_ps[key]
                nc.vector.memset(v_sb[tc_][:, :, HD], 1.0)
                for h in range(NHL):
                    nc.vector.tensor_add(
                        v_sb[tc_][:, h, 0:HD], ps[:, h * HD:(h + 1) * HD],
                        bvb[:, h * HD:(h + 1) * HD])

        # ---- prefix: scale, V, and Q/K for head pair 0 ----
        # own PSUM pool, released before the attention pools allocate
        with tc.tile_pool(name="prefpsum", bufs=4, space="PSUM") as pfp:
            sxc = ptmp.tile([GL, S], BF16, tag="sxc", bufs=1)
            for tb in range(NTB):
                ps = pfp.tile([P, NB], F32, tag="pp", name="pp")
                for k in range(KT):
                    nc.tensor.matmul(
                        ps[0:GL, :], ws_sb[k][:],
                        hsT[k][:, tb * NB:(tb + 1) * NB],
                        start=(k == 0), stop=(k == KT - 1))
                sg = ptmp.tile([GL, NB], F32, tag="sig")
                nc.scalar.activation(sg[:], ps[0:GL, :], AF.Sigmoid,
                                     bias=bs_sb[:])
                nc.vector.tensor_scalar(
                    sxc[:, tb * NB:(tb + 1) * NB], sg[:],
                    SC_MAX - SC_MIN, SC_MIN, ALU.mult, ALU.add)
            # replicate groups x8 into per-jb expanded scale tiles via DRAM
            # bounce (SBUF sources cannot partition-broadcast):
            nc.gpsimd.dma_start(sxd, sxc[:])
            for jb in range(JB):
                for hl in range(2):
                    for g in range(G):
                        src_row = 16 * jb + 8 * hl + g
                        nc.gpsimd.dma_start(
                            sexp[jb][hl * HD + G * g:hl * HD + G * g + G, :],
                            sxd[src_row:src_row + 1, :].broadcast_to([G, S]))
            # V projections (no sexp dependency -> overlap scale bounce)
            for tc_ in range(NTC):
                for half in range(2):
                    emit_v_unit(tc_, half, pfp, "pp")
            # pair-0 Q/K
            for kind in ("q", "k"):
                for tb in range(NTB):
                    for half in range(2):
                        emit_qk_unit(0, kind, tb, half, pfp, "pp")

        # deferred Q/K projection half-units for pairs 1..3. K first: its
        # free dim is t, read across the whole s-window from t-chunk 0, so
        # K(jb) must fully drain a window before pair jb starts; Q(jb) is
        # read per-s-window (tb 0,1 feed ssb 0; tb 2,3 feed ssb 1).
        projq = deque()
        for jb in range(1, JB):
            for kind in ("k", "q"):
                for tb in range(NTB):
                    for half in range(2):
                        projq.append((jb, kind, tb, half))

        # ================= attention =================
        # PSUM: sc [128,2048] = banks 0-3; cps0/cps1 [128,1024] = banks 4-7.
        # Filler units time-share the cps tags between windows.
        with tc.tile_pool(name="spsum", bufs=1, space="PSUM") as spool, \
             tc.tile_pool(name="cpsum", bufs=1, space="PSUM") as cpsum, \
             tc.tile_pool(name="ring", bufs=1) as rpool, \
             tc.tile_pool(name="asm", bufs=1) as apool, \
             tc.tile_pool(name="ctmp", bufs=2) as ctpool, \
             tc.tile_pool(name="trt", bufs=4) as trpool, \
             tc.tile_pool(name="rtmp", bufs=4) as rtpool:

            asm = [apool.tile([P, JL], F32, tag=f"asm{i}", name=f"asm{i}")
                   for i in range(S // P)]

            fill_i = [0]

            def drain_proj(n):
                # boundary filler: runs on the cps tags BETWEEN windows
                # (after the previous window's ctx drain, before the next
                # window's first ctx matmul) — the probs ring slack lets
                # scores/ACT stream while these occupy the ctx banks.
                for _ in range(n):
                    if not projq:
                        return
                    jb, kind, tb, half = projq.popleft()
                    emit_qk_unit(jb, kind, tb, half, cpsum,
                                 f"cps{fill_i[0] % 2}")
                    if half == 1:
                        fill_i[0] += 1

            ring = {}

            for hp in range(HP):
                for ssb in range(NSSB):
                    if (hp, ssb) != (0, 0):
                        drain_proj(8)
                    cps = [cpsum.tile([HD + 1, W2], F32, tag=f"cps{i}",
                                      name=f"cps{i}") for i in range(2)]

                    def ctx_chunk(tcc):
                        pts = ring.pop(tcc)
                        for i in range(2):
                            h = hp * 2 + i
                            for sh in range(W2 // NB):
                                nc.tensor.matmul(
                                    cps[i][0:HD + 1, sh * NB:(sh + 1) * NB],
                                    v_sb[tcc][:, h, :],
                                    pts[:, i * W2 + sh * NB:
                                        i * W2 + (sh + 1) * NB],
                                    start=(tcc == 0), stop=(tcc == NTC - 1))

                    for tc_ in range(NTC):
                        if tc_ >= LAG:
                            ctx_chunk(tc_ - LAG)
                        scol = ssb * W2
                        sc = spool.tile([P, 2 * W2], F32, tag="sc", name="sc")
                        for sbh in range(W2 // NB):
                            for i in range(2):
                                r0 = i * HD
                                nc.tensor.matmul(
                                    sc[:, i * W2 + sbh * NB:
                                       i * W2 + (sbh + 1) * NB],
                                    k_sb[hp][r0:r0 + HD, tc_ * P:(tc_ + 1) * P],
                                    q_sb[hp][r0:r0 + HD,
                                             scol + sbh * NB:
                                             scol + (sbh + 1) * NB],
                                    start=True, stop=True)
                        pts = rpool.tile([P, 2 * W2], BF16, tag="ring",
                                         bufs=RING, name="pts")
                        ring[tc_] = pts
                        # no max subtraction: |scores/8| < ~6
                        nc.scalar.activation(pts[:], sc[:], AF.Exp,
                                             scale=1.0 / 8.0)
                    for tcc in range(NTC - LAG, NTC):
                        ctx_chunk(tcc)

                    # normalize + transpose back per head: PSUM -> bf16 SBUF
                    # -> DMA-xbar transpose -> reciprocal-scaled fp32
                    for i in range(2):
                        h = hp * 2 + i
                        ct = ctpool.tile([P, W2], BF16, tag="ct")
                        nc.vector.tensor_copy(ct[0:HD + 1, :],
                                              cps[i][0:HD + 1, :])
                        for cc in range(W2 // P):
                            trc = trpool.tile([P, P], BF16, tag="tr")
                            nc.sync.dma_start_transpose(
                                trc[:], ct[:, cc * P:(cc + 1) * P])
                            rc = rtpool.tile([P, 1], F32, tag="rc")
                            nc.vector.reciprocal(rc[:], trc[:, HD:HD + 1])
                            nc.vector.tensor_scalar_mul(
                                asm[ssb * (W2 // P) + cc][:,
                                                          h * HD:(h + 1) * HD],
                                trc[:, 0:HD], rc[:])
                    if hp == HP - 1:
                        for i in range(W2 // P):
                            srow = ssb * W2 + i * P
                            nc.sync.dma_start(out[srow:srow + P, :],
                                              asm[ssb * (W2 // P) + i][:])

    nc.finalize()
    return nc


_CACHE = {}


def _get_nc():
    if "nc" not in _CACHE:
        _CACHE["nc"] = build_bass()
    return _CACHE["nc"]


def _shard(inputs):
    """Split full inputs into 8 per-core input maps (host-side, bf16 cast)."""
    hidden_states = inputs["hidden_states"]
    Wq, bq = inputs["Wq"], inputs["bq"]
    Wk, bk = inputs["Wk"], inputs["bk"]
    Wv, bv = inputs["Wv"], inputs["bv"]
    Ws, bs = inputs["Ws"], inputs["bs"]
    JL = 512   # output cols per core
    GL = 64    # Ws cols per core
    bf = ml_dtypes.bfloat16
    in_maps = []
    for c in range(8):
        b, hh = c // 2, c % 2
        in_maps.append({
            "hs": np.ascontiguousarray(hidden_states[b]).astype(bf),
            "wq": np.ascontiguousarray(Wq[:, hh * JL:(hh + 1) * JL]).astype(bf),
            "wk": np.ascontiguousarray(Wk[:, hh * JL:(hh + 1) * JL]).astype(bf),
            "wv": np.ascontiguousarray(Wv[:, hh * JL:(hh + 1) * JL]).astype(bf),
            "ws": np.ascontiguousarray(Ws[:, hh * GL:(hh + 1) * GL]).astype(bf),
            "bq": np.ascontiguousarray(bq[hh * JL:(hh + 1) * JL]).astype(np.float32),
            "bk": np.ascontiguousarray(bk[hh * JL:(hh + 1) * JL]).astype(np.float32),
            "bv": np.ascontiguousarray(bv[hh * JL:(hh + 1) * JL]).astype(np.float32),
            "bs": np.ascontiguousarray(bs[hh * GL:(hh + 1) * GL]).astype(np.float32),
        })
    return in_maps


def kernel(hidden_states, Wq, bq, Wk, bk, Wv, bv, Ws, bs):
    B, S, H = hidden_states.shape
    NH = 16
    JL = 512
    nc = _get_nc()
    in_maps = _shard(dict(hidden_states=hidden_states, Wq=Wq, bq=bq, Wk=Wk,
                          bk=bk, Wv=Wv, bv=bv, Ws=Ws, bs=bs))

    res = bass_utils.run_bass_kernel_spmd(nc, in_maps, core_ids=list(range(8)))

    outp = np.zeros((B, S, NH * HD), dtype=np.float32)
    for c in range(8):
        b, hh = c // 2, c % 2
        outp[b][:, hh * JL:(hh + 1) * JL] = res.results[c]["out"]
    return outp


# revision 14
# speedup vs baseline: 1.3423x; 1.3423x over previous
"""Trainium2 Bass kernel for CurvSelfAttention (v2).

Reference computation (per batch b):
    Q = hs @ Wq + bq ; K = hs @ Wk + bk ; V = hs @ Wv + bv      # [S, H]
    s = sigmoid(hs @ Ws + bs) * 0.2 + 0.9                        # [S, NH*G]
    Q[:, h*64+g*8+r] *= s[:, h*8+g]
    per head h: ctx_h = softmax(Q_h K_h^T / 8) V_h               # [S, 64]
    out = concat_h(ctx_h)                                        # [S, NH*64]

Sharding over 8 cores: core c = (b = c // 2, hh = c % 2); each core owns
batch b and heads hh*8 .. hh*8+8 (512 output columns). No collectives.

v2 schedule (vs v1): the kernel is ScalarE(exp)-bound, so everything is
arranged around one big 2048-wide ACT per (head-pair, t-chunk):
  - scores for BOTH heads of a pair land in one 4-bank PSUM tile
    [128, 2048] (quarters written by 4 matmuls, two concurrent per 64-row
    half of the PE array), drained by a single Exp ACTIVATE -> bf16 ring.
  - ctx accumulates [65, 1024] per head in the other 4 PSUM banks, lagging
    the probs ring by LAG chunks; V carries a ones column so row 64 is the
    softmax denominator.
  - ctx output is copied to bf16 and transposed by the DMA xbar engine
    (not TensorE), then normalized on VectorE (reciprocal of row 64).
  - Q/K projections for pairs 1-3 drain as filler units inside the
    attention loop (PSUM tags time-share the ctx banks); V + pair-0 Q/K +
    the dynamic scale run in a prefix whose PSUM pool is released before
    the attention pools allocate.
"""

import os
import sys

sys.path.insert(0, "/opt/trn_rl_repo")

import numpy as np
import ml_dtypes
from collections import deque
from contextlib import ExitStack

import concourse.bass as bass
import concourse.bacc as bacc
import concourse.tile as tile
from concourse import mybir
from concourse import bass_utils

F32 = mybir.dt.float32
BF16 = mybir.dt.bfloat16
AF = mybir.ActivationFunctionType
ALU = mybir.AluOpType

P = 128          # SBUF partitions
NB = 512         # matmul moving free-dim block
W2 = 1024        # attention s-window per head (one ctx psum row)
HD = 64          # head dim
G = 8            # groups per head
RING = 8         # probs ring chunks
LAG = 2          # ctx trails exp by this many chunks (no boundary work)
LAGB = 8         # ctx delay on windows that absorb boundary tasks
SC_MIN, SC_MAX = 0.9, 1.1


def build_bass(S=2048, H=1024, NHL=8):
    """Build the per-core Bass module. NHL = local heads; JL = NHL*64."""
    JL = NHL * HD
    GL = NHL * G           # compact scale channels
    KT = H // P            # contraction k-tiles
    JB = JL // P           # j row-blocks for Q/K (2 heads each)
    NTB = S // NB          # 512-wide t blocks
    NTC = S // P           # 128-wide t chunks
    NSSB = S // W2         # s-windows per head
    HP = NHL // 2          # head pairs

    nc = bacc.Bacc(trn_type="TRN2", target_bir_lowering=False, debug=False,
                   num_devices=8)

    hs = nc.dram_tensor("hs", [S, H], BF16, kind="ExternalInput").ap()
    wq = nc.dram_tensor("wq", [H, JL], BF16, kind="ExternalInput").ap()
    wk = nc.dram_tensor("wk", [H, JL], BF16, kind="ExternalInput").ap()
    wv = nc.dram_tensor("wv", [H, JL], BF16, kind="ExternalInput").ap()
    ws = nc.dram_tensor("ws", [H, GL], BF16, kind="ExternalInput").ap()
    bq = nc.dram_tensor("bq", [JL], F32, kind="ExternalInput").ap()
    bk = nc.dram_tensor("bk", [JL], F32, kind="ExternalInput").ap()
    bv = nc.dram_tensor("bv", [JL], F32, kind="ExternalInput").ap()
    bs = nc.dram_tensor("bs", [GL], F32, kind="ExternalInput").ap()
    ident = nc.dram_tensor("ident", [P, P], BF16, kind="ExternalInput").ap()
    out = nc.dram_tensor("out", [S, JL], F32, kind="ExternalOutput").ap()
    sxd = nc.dram_tensor("sxd", [GL, S], BF16, kind="Internal").ap()

    with tile.TileContext(nc) as tc, ExitStack() as ctx:
        cpool = ctx.enter_context(tc.tile_pool(name="consts", bufs=1))
        qkpool = ctx.enter_context(tc.tile_pool(name="qk", bufs=1))
        vpool = ctx.enter_context(tc.tile_pool(name="v", bufs=1))
        sxpool = ctx.enter_context(tc.tile_pool(name="sexp", bufs=1))
        hpool = ctx.enter_context(tc.tile_pool(name="hsT", bufs=1))
        wpool = ctx.enter_context(tc.tile_pool(name="wts", bufs=1))
        ptmp = ctx.enter_context(tc.tile_pool(name="ptmp", bufs=2))

        # ---- constants (small DMAs on the SW DGE queue) ----
        bq_sb = []
        bk_sb = []
        for jb in range(JB):
            t = cpool.tile([P, 1], F32, tag=f"bq{jb}")
            nc.gpsimd.dma_start(
                t[:], bq[jb * P:(jb + 1) * P].rearrange("(a b) -> a b", b=1))
            bq_sb.append(t)
            t = cpool.tile([P, 1], F32, tag=f"bk{jb}")
            nc.gpsimd.dma_start(
                t[:], bk[jb * P:(jb + 1) * P].rearrange("(a b) -> a b", b=1))
            bk_sb.append(t)
        bs_sb = cpool.tile([GL, 1], F32, tag="bs")
        nc.gpsimd.dma_start(bs_sb[:], bs.rearrange("(a b) -> a b", b=1))
        bvb = cpool.tile([P, JL], F32, tag="bvb")
        nc.gpsimd.dma_start(
            bvb[:], bv.rearrange("(a b) -> a b", a=1).broadcast_to([P, JL]))
        idf = cpool.tile([P, P], BF16, tag="idf")
        nc.gpsimd.dma_start(idf[:], ident)

        # persistent activation tensors
        q_sb = [qkpool.tile([P, S], BF16, tag=f"q{jb}", name=f"q{jb}")
                for jb in range(JB)]
        k_sb = [qkpool.tile([P, S], BF16, tag=f"k{jb}", name=f"k{jb}")
                for jb in range(JB)]
        # V as [t-chunk][128, NHL, 65]; col 64 of each head = ones (denom)
        v_sb = [vpool.tile([P, NHL, HD + 1], BF16, tag=f"v{tc_}", name=f"v{tc_}")
                for tc_ in range(NTC)]
        sexp = [sxpool.tile([P, S], BF16, tag=f"sx{jb}", name=f"sx{jb}")
                for jb in range(JB)]

        # ---- input loads: ws first, hsT transposes, then weights ----
        ws_sb = []
        for k in range(KT):
            t = wpool.tile([P, GL], BF16, tag=f"ws{k}", name=f"ws{k}")
            nc.sync.dma_start(t[:], ws[k * P:(k + 1) * P, :])
            ws_sb.append(t)
        hsT = []
        for k in range(KT):
            t = hpool.tile([P, S], BF16, tag=f"hsT{k}", name=f"hsT{k}")
            nc.sync.dma_start_transpose(t[:], hs[:, k * P:(k + 1) * P])
            hsT.append(t)
        wq_sb, wk_sb, wv_sb = [], [], []
        for k in range(KT):
            for name, dram, lst in (("wv", wv, wv_sb), ("wq", wq, wq_sb),
                                    ("wk", wk, wk_sb)):
                t = wpool.tile([P, JL], BF16, tag=f"{name}{k}",
                               name=f"{name}{k}")
                nc.sync.dma_start(t[:], dram[k * P:(k + 1) * P, :])
                lst.append(t)

        # ---- projection work units ----
        # deferred Q/K half-units (4 matmuls each) keyed for the attention
        # loop's filler slots; V and pair-0 Q/K run in the prefix.
        open_ps = {}

        def emit_qk_unit(jb, kind, tb, half, pool, tag):
            wlist = wq_sb if kind == "q" else wk_sb
            key = (jb, kind, tb)
            if half == 0:
                open_ps[key] = pool.tile([P, NB], F32, tag=tag, name="pp")
            ps = open_ps[key]
            kh = KT // 2
            for k in range(kh * half, kh * half + kh):
                nc.tensor.matmul(
                    ps[:], wlist[k][:, jb * P:(jb + 1) * P],
                    hsT[k][:, tb * NB:(tb + 1) * NB],
                    start=(k == 0), stop=(k == KT - 1))
            if half == 1:
                del open_ps[key]
                if kind == "q":
                    nc.vector.scalar_tensor_tensor(
                        q_sb[jb][:, tb * NB:(tb + 1) * NB], ps[:],
                        bq_sb[jb][:], sexp[jb][:, tb * NB:(tb + 1) * NB],
                        ALU.add, ALU.mult)
                else:
                    nc.vector.tensor_scalar_add(
                        k_sb[jb][:, tb * NB:(tb + 1) * NB], ps[:], bk_sb[jb][:])

        def emit_v_unit(tc_, half, pool, tag):
            key = ("v", tc_)
            if half == 0:
                open_ps[key] = pool.tile([P, JL], F32, tag=tag, name="pp")
            ps = open_ps[key]
            kh = KT // 2
            for k in range(kh * half, kh * half + kh):
                nc.tensor.matmul(
                    ps[:], hsT[k][:, tc_ * P:(tc_ + 1) * P], wv_sb[k][:],
                    start=(k == 0), stop=(k == KT - 1))
            if half == 1:
                del open_ps[key]
                nc.vector.memset(v_sb[tc_][:, :, HD], 1.0)
                for h in range(NHL):
                    nc.vector.tensor_add(
                        v_sb[tc_][:, h, 0:HD], ps[:, h * HD:(h + 1) * HD],
                        bvb[:, h * HD:(h + 1) * HD])

        # ---- prefix: scale, V, and Q/K for head pair 0 ----
        # own PSUM pool, released before the attention pools allocate
        with tc.tile_pool(name="prefpsum", bufs=4, space="PSUM") as pfp:
            sxc = ptmp.tile([GL, S], BF16, tag="sxc", bufs=1)
            for tb in range(NTB):
                ps = pfp.tile([P, NB], F32, tag="pp", name="pp")
                for k in range(KT):
                    nc.tensor.matmul(
                        ps[0:GL, :], ws_sb[k][:],
                        hsT[k][:, tb * NB:(tb + 1) * NB],
                        start=(k == 0), stop=(k == KT - 1))
                sg = ptmp.tile([GL, NB], F32, tag="sig")
                nc.scalar.activation(sg[:], ps[0:GL, :], AF.Sigmoid,
                                     bias=bs_sb[:])
                nc.vector.tensor_scalar(
                    sxc[:, tb * NB:(tb + 1) * NB], sg[:],
                    SC_MAX - SC_MIN, SC_MIN, ALU.mult, ALU.add)
            # replicate groups x8 into per-jb expanded scale tiles via DRAM
            # bounce (SBUF sources cannot partition-broadcast):
            nc.gpsimd.dma_start(sxd, sxc[:])
            for jb in range(JB):
                for hl in range(2):
                    for g in range(G):
                        src_row = 16 * jb + 8 * hl + g
                        nc.gpsimd.dma_start(
                            sexp[jb][hl * HD + G * g:hl * HD + G * g + G, :],
                            sxd[src_row:src_row + 1, :].broadcast_to([G, S]))
            # V projections (no sexp dependency -> overlap scale bounce)
            for tc_ in range(NTC):
                for half in range(2):
                    emit_v_unit(tc_, half, pfp, "pp")
            # pair-0 Q/K
            for kind in ("q", "k"):
                for tb in range(NTB):
                    for half in range(2):
                        emit_qk_unit(0, kind, tb, half, pfp, "pp")

        # deferred Q/K projection half-units for pairs 1..3, ordered so each
        # block is EMITTED at least one window before its first reader
        # (Tile deps come from program order; a later write is a race):
        #   window (jb-1, 1) drains q tb0,1 + k tb0,1 -> read from (jb, 0)
        #   window (jb, 0)  drains k tb2,3 (read at tc>=8 same window) +
        #                   q tb2,3 (read from (jb, 1))
        projq = deque()
        for jb in range(1, JB):
            for kind, tb in (("q", 0), ("q", 1), ("k", 0), ("k", 1),
                             ("k", 2), ("k", 3), ("q", 2), ("q", 3)):
                for half in range(2):
                    projq.append((jb, kind, tb, half))

        # ================= attention =================
        # PSUM: scA, scB [128,1024] ping-pong = banks 0-3; cps0/cps1
        # [128,1024] = banks 4-7. Filler units and the previous window's
        # transpose scratch time-share the cps tags between a window's ctx
        # drain and the next window's (delayed) first ctx matmul.
        with tc.tile_pool(name="spsum", bufs=1, space="PSUM") as spool, \
             tc.tile_pool(name="cpsum", bufs=1, space="PSUM") as cpsum, \
             tc.tile_pool(name="ring", bufs=1) as rpool, \
             tc.tile_pool(name="asm", bufs=1) as apool, \
             tc.tile_pool(name="ctmp", bufs=2) as ctpool, \
             tc.tile_pool(name="rtmp", bufs=4) as rtpool:

            asm = [apool.tile([P, JL], F32, tag=f"asm{i}", name=f"asm{i}")
                   for i in range(S // P)]

            fill_i = [0]

            def filler_task():
                if not projq:
                    return False
                jb, kind, tb, half = projq.popleft()
                emit_qk_unit(jb, kind, tb, half, cpsum, f"cps{fill_i[0] % 2}")
                if half == 1:
                    fill_i[0] += 1
                return True

            def make_post_tasks(hp, ssb, cts):
                """Transpose+normalize closures for a finished window.

                Each task handles one 128-column chunk of one head: PE
                transpose (bf16) into a scratch tile on a cps tag, then
                reciprocal-of-denominator scale into asm.  Runs during the
                NEXT window's early iterations, before its ctx starts.
                """
                trp_tiles = {}
                tasks = []

                def task(i, cc, emit_dma):
                    def run():
                        if i not in trp_tiles:
                            # chunk stride padded to 66 (132 B) so each
                            # PSUM write stays 4-byte aligned
                            trp_tiles[i] = cpsum.tile(
                                [P, W2 // P, HD + 2], BF16, tag=f"cps{i}",
                                name="trp")
                        trp = trp_tiles[i]
                        h = hp * 2 + i
                        nc.tensor.transpose(
                            trp[:, cc, 0:HD + 1],
                            cts[i][0:HD + 1, cc * P:(cc + 1) * P],
                            idf[0:HD + 1, 0:HD + 1])
                        rc = rtpool.tile([P, 1], F32, tag="rc")
                        nc.vector.reciprocal(rc[:], trp[:, cc, HD:HD + 1])
                        nc.vector.tensor_scalar_mul(
                            asm[ssb * (W2 // P) + cc][:, h * HD:(h + 1) * HD],
                            trp[:, cc, 0:HD], rc[:])
                        if emit_dma:
                            srow = ssb * W2 + cc * P
                            nc.sync.dma_start(
                                out[srow:srow + P, :],
                                asm[ssb * (W2 // P) + cc][:])
                    return run

                for cc in range(W2 // P):
                    for i in range(2):
                        tasks.append(task(i, cc, hp == HP - 1 and i == 1))
                return tasks

            ring = {}
            post_tasks = []

            for hp in range(HP):
                for ssb in range(NSSB):
                    first = (hp, ssb) == (0, 0)
                    boundary = deque(post_tasks)
                    post_tasks = []
                    lag0 = LAG if first else LAGB

                    cps = []

                    def ensure_cps():
                        # allocated lazily so the boundary tasks' psum tiles
                        # precede these in the cps tag rotation
                        if not cps:
                            for i in range(2):
                                cps.append(cpsum.tile(
                                    [HD + 1, W2], F32, tag=f"cps{i}",
                                    name=f"cps{i}"))

                    def ctx_chunk(tcc):
                        ensure_cps()
                        pts = ring.pop(tcc)
                        for i in range(2):
                            h = hp * 2 + i
                            for sh in range(2):
                                nc.tensor.matmul(
                                    cps[i][0:HD + 1, sh * NB:(sh + 1) * NB],
                                    v_sb[tcc][:, h, :],
                                    pts[:, sh, i, :],
                                    start=(tcc == 0), stop=(tcc == NTC - 1))

                    ctx_done = [0]

                    def ctx_upto(limit):
                        while ctx_done[0] < min(limit, NTC):
                            ctx_chunk(ctx_done[0])
                            ctx_done[0] += 1

                    scol = ssb * W2
                    for tc_ in range(NTC):
                        # boundary work: filler projections at tc 0-3,
                        # prev-window transpose/normalize at tc 4-7
                        if not first:
                            if tc_ < 4:
                                filler_task()
                                filler_task()
                            elif tc_ < 8:
                                for _ in range(4):
                                    if boundary:
                                        boundary.popleft()()
                        # ctx: simple lag when no boundary work, else
                        # catch-up from tc 8 at ~9/4 chunks per iteration
                        if tc_ >= lag0:
                            if lag0 == LAG:
                                ctx_upto(tc_ - LAG + 1)
                            else:
                                ctx_upto(min(tc_ - 1, (tc_ - 7) * 9 // 4))
                        pts = rpool.tile([P, 2, 2, NB], BF16, tag="ring",
                                         bufs=RING, name="pts")
                        ring[tc_] = pts
                        for sbh in range(2):
                            sct = spool.tile([P, W2], F32,
                                             tag=f"sc{sbh}", name="sct")
                            for i in range(2):
                                r0 = i * HD
                                nc.tensor.matmul(
                                    sct[:, i * NB:(i + 1) * NB],
                                    k_sb[hp][r0:r0 + HD, tc_ * P:(tc_ + 1) * P],
                                    q_sb[hp][r0:r0 + HD,
                                             scol + sbh * NB:
                                             scol + (sbh + 1) * NB],
                                    start=True, stop=True)
                            # no max subtraction: |scores/8| < ~6
                            nc.scalar.activation(pts[:, sbh], sct[:], AF.Exp,
                                                 scale=1.0 / 8.0)
                    while boundary:
                        boundary.popleft()()
                    ctx_upto(NTC)

                    # drain ctx accumulators to bf16 SBUF; transposes run
                    # as the next window's boundary tasks
                    cts = []
                    for i in range(2):
                        ct = ctpool.tile([P, W2], BF16, tag=f"ct{i}",
                                         name="ct")
                        nc.vector.tensor_copy(ct[0:HD + 1, :],
                                              cps[i][0:HD + 1, :])
                        cts.append(ct)
                    post_tasks = make_post_tasks(hp, ssb, cts)

            # final window's transposes + output
            for t in post_tasks:
                t()

    nc.finalize()
    return nc


_CACHE = {}


def _get_nc():
    if "nc" not in _CACHE:
        _CACHE["nc"] = build_bass()
    return _CACHE["nc"]


def _shard(inputs):
    """Split full inputs into 8 per-core input maps (host-side, bf16 cast)."""
    hidden_states = inputs["hidden_states"]
    Wq, bq = inputs["Wq"], inputs["bq"]
    Wk, bk = inputs["Wk"], inputs["bk"]
    Wv, bv = inputs["Wv"], inputs["bv"]
    Ws, bs = inputs["Ws"], inputs["bs"]
    JL = 512   # output cols per core
    GL = 64    # Ws cols per core
    bf = ml_dtypes.bfloat16
    ident = np.eye(P, dtype=np.float32).astype(bf)
    in_maps = []
    for c in range(8):
        b, hh = c // 2, c % 2
        in_maps.append({
            "hs": np.ascontiguousarray(hidden_states[b]).astype(bf),
            "wq": np.ascontiguousarray(Wq[:, hh * JL:(hh + 1) * JL]).astype(bf),
            "wk": np.ascontiguousarray(Wk[:, hh * JL:(hh + 1) * JL]).astype(bf),
            "wv": np.ascontiguousarray(Wv[:, hh * JL:(hh + 1) * JL]).astype(bf),
            "ws": np.ascontiguousarray(Ws[:, hh * GL:(hh + 1) * GL]).astype(bf),
            "bq": np.ascontiguousarray(bq[hh * JL:(hh + 1) * JL]).astype(np.float32),
            "bk": np.ascontiguousarray(bk[hh * JL:(hh + 1) * JL]).astype(np.float32),
            "bv": np.ascontiguousarray(bv[hh * JL:(hh + 1) * JL]).astype(np.float32),
            "bs": np.ascontiguousarray(bs[hh * GL:(hh + 1) * GL]).astype(np.float32),
            "ident": ident,
        })
    return in_maps


def kernel(hidden_states, Wq, bq, Wk, bk, Wv, bv, Ws, bs):
    B, S, H = hidden_states.shape
    NH = 16
    JL = 512
    nc = _get_nc()
    in_maps = _shard(dict(hidden_states=hidden_states, Wq=Wq, bq=bq, Wk=Wk,
                          bk=bk, Wv=Wv, bv=bv, Ws=Ws, bs=bs))

    res = bass_utils.run_bass_kernel_spmd(nc, in_maps, core_ids=list(range(8)))

    outp = np.zeros((B, S, NH * HD), dtype=np.float32)
    for c in range(8):
        b, hh = c // 2, c % 2
        outp[b][:, hh * JL:(hh + 1) * JL] = res.results[c]["out"]
    return outp
